# revision 29
# baseline (speedup 1.0000x reference)
"""Trainium2 Bass kernel for nn_Attention (dense transformer block, full-dim attention).

Reference computation (per batch b):
    qn/kn/vn = LayerNorm(q/k/v[b])           # over C=256
    qp = qn @ Wq + bq; kp = kn @ Wk + bk; vp = vn @ Wv + bv   # [N, 1024]
    S  = qp @ kp.T * 64^-0.5; P = softmax(S); out = (P @ vp) @ Wo + bo

Key observation: the INNER=1024 dim only appears inside two weight-weight
products, so the whole block is rank-256 through the attention:
    S   = x^q (Wq' Wk'^T) x^k.T + [q-only] + w_k + [const]
    out = P x^v (Wv' Wo) / rowsum + bo''
where x^ = (x-mu)*rstd (pure LN), Wq' = diag(g)Wq etc. Host precomputes
    M = Wq' Wk'^T  [256,256]     U = Wv' Wo  [256,256]
    v0 = SCALE * Wk' @ (beta Wq + bq)        (the k-dependent bias row)
    bo'' = bo + (beta Wv + bv) @ Wo          (exact: softmax rows sum to 1)
The q-only and constant S terms cancel in softmax and are dropped; w_k
rides the ACT exp eviction as a per-partition bias. The 1024-dim
projections, Wq/Wk/Wv/Wo streaming, and their SBUF residency all vanish:
per-core matmul work drops from ~944 to ~256 instructions.

Sharding: 8 cores = 4 batches x 2 query-row halves; k/v LN is duplicated
within the pair (no projections left to dedup -- not worth a collective).

On-chip dataflow (per core), contraction on partitions, operands bf16:
    x^qT, x^kT : LN stats natural (batched bn_stats), PE-transpose
    x^v        : LN apply straight to natural bf16 (no transpose)
    AT  [c',q] : M chunks (lhsT) x x^qT          (8 MMs)
    w   [1,N]  : v0 1-col chunks (lhsT) x x^kT   (8 MMs), DRAM-bounce
                 scatter to [128, MT] per-partition layout
    S^T        : x^kT tiles (lhsT) x AT          (64 MMs, FD=512)
    expS^T     : ACT exp(0.125*S^T + w_m) from PSUM, bf16
    rowsum     : ones [128,128] (lhsT) x expS -> every partition holds the
                 k-sum; reciprocal in free-dim layout      (32 MMs)
    Y^T [c,q]  : x^v tiles (lhsT) x expS^T       (64 MMs)
    out^T      : U chunks (lhsT) x Y^T           (8 MMs), eviction
                 multiplies by recipF (per-free) and adds bo'' (per-part);
                 out stored transposed [C, NQ], host transposes back.

Phase order keeps all ACT Sqrt ops before all Exp ops (activation table
switches cost ~2.7us). A short identity-transpose burst at kernel start
bridges the LN startup bubble and keeps the PE HAM clock gate released.
A post-scheduling pass splits multi-wait instructions (this walrus's
instruction structs carry at most 1-2 sync waits)."""

import numpy as np
import ml_dtypes

import concourse.bass as bass
import concourse.tile as tile
from concourse import mybir
from concourse.bass_utils import run_bass_kernel_spmd

# Problem shapes (hardcoded per contract)
B = 4
N = 2048          # sequence length (k/v tokens per core)
C = 256           # channels
NQ = 1024         # query rows per core (N/2)
EPS = 1e-5
SCALE = 0.125     # 64 ** -0.5
P = 128

FP = mybir.dt.float32
BF = mybir.dt.bfloat16

NCORES = 8
CCH = C // P          # 2 chunks of the channel dim
MT = N // P           # 16 k-token tiles
QT = NQ // P          # 8 q-token tiles
QCH = NQ // 512       # 2 q-token free chunks
KCH = N // 512        # 4 k-token free chunks

_sub = mybir.AluOpType.subtract
_mult = mybir.AluOpType.mult

WARM = 24  # PE warm-up transposes bridging the LN startup bubble


def _bcast(ap, parts=P):
    # prepend a stride-0 partition dim: [n] -> [parts, n]
    return bass.AP(tensor=ap.tensor, offset=ap.offset,
                   ap=[[0, parts]] + [list(d) for d in ap.ap])


def _emit_consts(nc, tc, ctx, io):
    consts = ctx.enter_context(tc.tile_pool(name="consts", bufs=1))
    # ---- constants (scalar/gpsimd DMA queues: off the load path) --
    # M/U as stationary chunks: [128 (contraction part), chunk, out-cols]
    M_sb = consts.tile([P, CCH, C], BF)
    nc.scalar.dma_start(M_sb, io["M"].rearrange("(c p) n -> p c n", p=P))
    U_sb = consts.tile([P, CCH, C], BF)
    nc.scalar.dma_start(U_sb, io["U"].rearrange("(c p) n -> p c n", p=P))
    v0_sb = consts.tile([P, CCH], BF)
    nc.scalar.dma_start(v0_sb, io["v0"].rearrange("(c p) -> p c", p=P))
    ones1 = consts.tile([P, 1], BF)
    nc.vector.memset(ones1, 1.0)
    eps_sb = consts.tile([P, 1], FP)
    nc.vector.memset(eps_sb, EPS)
    ident = consts.tile([P, P], BF)
    from concourse.masks import make_identity
    make_identity(nc, ident)
    # bo as a 1-partition row (bf16): rank-1 matmul folds the bias into
    # the out^T accumulation pre-normalization (bo * rowsum)
    b_row = consts.tile([1, C], BF)
    nc.gpsimd.dma_start(b_row, io["bo"])
    return dict(M_sb=M_sb, U_sb=U_sb, v0_sb=v0_sb, ones1=ones1,
                eps_sb=eps_sb, ident=ident, b_row=b_row)


def _emit(nc, tc, io, cst):
    from contextlib import ExitStack

    M_sb = cst["M_sb"]; U_sb = cst["U_sb"]; v0_sb = cst["v0_sb"]
    ones1 = cst["ones1"]; eps_sb = cst["eps_sb"]; ident = cst["ident"]
    b_row = cst["b_row"]

    with ExitStack() as ctx:
        big = ctx.enter_context(tc.tile_pool(name="big", bufs=1))
        ln_pool = ctx.enter_context(tc.tile_pool(name="ln", bufs=4))
        stat = ctx.enter_context(tc.tile_pool(name="stat", bufs=4))
        temps = ctx.enter_context(tc.tile_pool(name="temps", bufs=3))
        psum = ctx.enter_context(tc.tile_pool(name="psum", bufs=4, space="PSUM"))
        psum_rs = ctx.enter_context(tc.tile_pool(name="psum_rs", bufs=1, space="PSUM"))
        psum_t = ctx.enter_context(tc.tile_pool(name="psum_t", bufs=2, space="PSUM"))

        # ---- persistent activations ----------------------------------
        xqT = big.tile([P, CCH, NQ], BF)
        xkT = big.tile([P, CCH, N], BF)
        xv = big.tile([P, MT, C], BF)
        AT = big.tile([P, CCH, NQ], BF)
        YT = big.tile([P, CCH, NQ], BF)
        expS = big.tile([P, MT, NQ], BF)
        recipF = big.tile([P, NQ], FP)
        recip1 = big.tile([1, NQ], FP)
        rs_sb = big.tile([1, QCH, 512], BF)
        wm_sb = big.tile([P, MT], FP)
        # batched input staging: 4 token-tiles per DMA (amortizes the
        # ~0.65us per-DMA issue cost that dominated the LN front latency)
        xq_l = big.tile([P, QT, C], FP)
        xk_l = big.tile([P, MT, C], FP)
        xv_l = big.tile([P, MT, C], FP)
        for src, dst, nt in ((io["xq"], xq_l, QT), (io["xk"], xk_l, MT),
                             (io["xv"], xv_l, MT)):
            r4 = src.rearrange("(t p) c -> p t c", p=P)
            for t0 in range(0, nt, 4):
                nc.sync.dma_start(dst[:, t0:t0 + 4, :], r4[:, t0:t0 + 4, :])

        # PE warm-up during the LN-chain startup bubble: sustained activity
        # releases the HAM clock gate (1.2 -> 2.4 GHz) before real matmuls
        warm = psum_t.tile([P, P], BF, tag="pst", name="warm")
        for w in range(WARM):
            nc.tensor.transpose(warm, ident, ident)

        # ---- layernorm: stats in natural layout, batched -------------
        def layernorm(x_l, ntiles, dstT=None, dst_nat=None):
            # groups of 8 tiles: batched stats -> one sqrt/recip per group.
            # dstT: apply + PE-transpose (plain eviction, gamma/beta live in
            # the host-folded weights). dst_nat: apply straight to bf16.
            for g0 in range(0, ntiles, 8):
                gn = min(8, ntiles - g0)
                mv_g = stat.tile([P, 8, 2], FP, tag="mv_g")
                xts = []
                for ii in range(gn):
                    i = g0 + ii
                    xt = x_l[:, i, :]
                    st = stat.tile([P, 6], FP, tag="st")
                    nc.vector.bn_stats(st, xt)
                    nc.vector.bn_aggr(mv_g[:, ii, :], st)
                    xts.append(xt)
                rstd_g = stat.tile([P, 8], FP, tag="rstd_g")
                nc.scalar.activation(rstd_g[:, :gn], mv_g[:, :gn, 1],
                                     mybir.ActivationFunctionType.Sqrt,
                                     bias=eps_sb, scale=1.0)
                nc.vector.reciprocal(rstd_g[:, :gn], rstd_g[:, :gn])
                # negated mu*rstd so half the applies can ride ACT's
                # scale/bias path: x*rstd + (-mu*rstd)
                nmr_g = stat.tile([P, 8], FP, tag="nmr_g")
                nc.vector.scalar_tensor_tensor(nmr_g[:, :gn], mv_g[:, :gn, 0],
                                               -1.0, rstd_g[:, :gn],
                                               op0=_mult, op1=_mult)
                for ii in range(gn):
                    i = g0 + ii
                    if dst_nat is not None:
                        dst = dst_nat[:, i, :]
                        if ii % 2 == 0:
                            nc.vector.tensor_scalar(dst, xts[ii],
                                                    mv_g[:, ii, 0:1],
                                                    rstd_g[:, ii:ii + 1],
                                                    op0=_sub, op1=_mult)
                        else:
                            nc.scalar.activation(
                                dst, xts[ii],
                                mybir.ActivationFunctionType.Identity,
                                bias=nmr_g[:, ii:ii + 1],
                                scale=rstd_g[:, ii:ii + 1])
                        continue
                    xn = ln_pool.tile([P, C], BF, tag="xn")
                    if ii % 2 == 0:
                        nc.vector.tensor_scalar(xn, xts[ii], mv_g[:, ii, 0:1],
                                                rstd_g[:, ii:ii + 1],
                                                op0=_sub, op1=_mult)
                    else:
                        nc.scalar.activation(
                            xn, xts[ii],
                            mybir.ActivationFunctionType.Identity,
                            bias=nmr_g[:, ii:ii + 1],
                            scale=rstd_g[:, ii:ii + 1])
                    for c in range(CCH):
                        pst = psum_t.tile([P, P], BF, tag="pst")
                        nc.tensor.transpose(pst, xn[:, c * P:(c + 1) * P],
                                            ident)
                        dst = dstT[:, c, i * P:(i + 1) * P]
                        if (i + c) % 2 == 0:
                            nc.vector.tensor_copy(dst, pst)
                        else:
                            nc.scalar.copy(dst, pst)

        # ---- phase 1: LN(q), A^T = M-chunks x x^qT -------------------
        layernorm(xq_l, QT, dstT=xqT)
        for jp in range(CCH):
            for n in range(QCH):
                ps = psum.tile([P, 512], FP, tag="ps")
                for cc in range(CCH):
                    nc.tensor.matmul(ps,
                                     lhsT=M_sb[:, cc, jp * P:(jp + 1) * P],
                                     rhs=xqT[:, cc, n * 512:(n + 1) * 512],
                                     start=(cc == 0), stop=(cc == CCH - 1))
                d = AT[:, jp, n * 512:(n + 1) * 512]
                if (jp + n) % 2 == 0:
                    nc.vector.tensor_copy(d, ps)
                else:
                    nc.scalar.copy(d, ps)

        # ---- phase 2: LN(k), LN(v) -----------------------------------
        layernorm(xk_l, MT, dstT=xkT)
        layernorm(xv_l, MT, dst_nat=xv)
        # (all ACT Sqrt ops are now done -- Exp table loads next)

        # ---- phase 3: w column, S^T, exp -----------------------------
        # w = x^k @ v0 (k-dependent exp bias) computed directly in the
        # per-partition layout the exp bias needs: x^kT tiles as stationary,
        # v0 chunk as a 1-wide moving operand -> wm[128 k, m] in one bank
        wm_ps = psum_rs.tile([P, MT], FP, tag="wmps")
        for m in range(MT):
            for cc in range(CCH):
                nc.tensor.matmul(wm_ps[:, m:m + 1],
                                 lhsT=xkT[:, cc, m * P:(m + 1) * P],
                                 rhs=v0_sb[:, cc:cc + 1],
                                 start=(cc == 0), stop=(cc == CCH - 1))
        nc.vector.tensor_copy(wm_sb, wm_ps)

        for m in range(MT):
            for n in range(QCH):
                ps = psum.tile([P, 512], FP, tag="ps")
                for cc in range(CCH):
                    nc.tensor.matmul(ps,
                                     lhsT=xkT[:, cc, m * P:(m + 1) * P],
                                     rhs=AT[:, cc, n * 512:(n + 1) * 512],
                                     start=(cc == 0), stop=(cc == CCH - 1))
                nc.scalar.activation(expS[:, m, n * 512:(n + 1) * 512], ps,
                                     mybir.ActivationFunctionType.Exp,
                                     bias=wm_sb[:, m:m + 1], scale=SCALE)

        # ---- phase 4: rowsums + Y^T = x^v-tiles x expS^T -------------
        # rowsums: 1-col ones stationary (LDW ~free), result on partition 0;
        # the two q-chunks share one PSUM bank, with Y matmul groups
        # interleaved so the bank's WAR wait (reciprocal eviction) is hidden
        def rowsum(n):
            rsb = psum_rs.tile([1, 512], FP, tag="rs", name="rsb")
            for m in range(MT):
                nc.tensor.matmul(rsb,
                                 lhsT=ones1,
                                 rhs=expS[:, m, n * 512:(n + 1) * 512],
                                 start=(m == 0), stop=(m == MT - 1))
            nc.vector.reciprocal(recip1[:, n * 512:(n + 1) * 512], rsb)
            nc.scalar.copy(rs_sb[:, n, :], rsb)
            # broadcast the reciprocal row to all partitions (DRAM bounce);
            # runs during the Y matmuls, ready before the out^T evictions
            nc.sync.dma_start(io["rscr"][n * 512:(n + 1) * 512],
                              recip1[0:1, n * 512:(n + 1) * 512])
            nc.gpsimd.dma_start(
                recipF[:, n * 512:(n + 1) * 512],
                _bcast(io["rscr"][n * 512:(n + 1) * 512]))

        def ygroup(j, n):
            ps = psum.tile([P, 512], FP, tag="ps")
            for m in range(MT):
                nc.tensor.matmul(ps,
                                 lhsT=xv[:, m, j * P:(j + 1) * P],
                                 rhs=expS[:, m, n * 512:(n + 1) * 512],
                                 start=(m == 0), stop=(m == MT - 1))
            d = YT[:, j, n * 512:(n + 1) * 512]
            if (j + n) % 2 == 0:
                nc.vector.tensor_copy(d, ps)
            else:
                nc.scalar.copy(d, ps)

        # out^T = U-chunks x Y^T; a rank-1 matmul adds bo * rowsum inside
        # the accumulation (division by rowsum then yields +bo exactly), so
        # the eviction is a single per-free multiply by recipF
        def outgroup(ci, n):
            ps = psum.tile([P, 512], FP, tag="ps")
            for cc in range(CCH):
                nc.tensor.matmul(ps,
                                 lhsT=U_sb[:, cc, ci * P:(ci + 1) * P],
                                 rhs=YT[:, cc, n * 512:(n + 1) * 512],
                                 start=(cc == 0), stop=False)
            nc.tensor.matmul(ps,
                             lhsT=b_row[:, ci * P:(ci + 1) * P],
                             rhs=rs_sb[:, n, :],
                             start=False, stop=True)
            o1 = temps.tile([P, 512], FP, tag="o1")
            nc.vector.tensor_tensor(o1, ps,
                                    recipF[:, n * 512:(n + 1) * 512],
                                    _mult)
            dma = nc.sync if (ci + n) % 2 == 0 else nc.scalar
            dma.dma_start(
                io["out"][ci * P:(ci + 1) * P, n * 512:(n + 1) * 512], o1)

        # interleave: out(*, n=0) runs while Y(*, n=1) streams on the PE
        rowsum(0)
        ygroup(0, 0)
        rowsum(1)
        ygroup(1, 0)
        outgroup(0, 0)
        outgroup(1, 0)
        ygroup(0, 1)
        ygroup(1, 1)
        outgroup(0, 1)
        outgroup(1, 1)


_DMA_WAIT_LIMIT = 1
_ENGINE_WAIT_LIMIT = 1


def _split_dma_waits(nc, wsem):
    """This walrus's instruction structs carry very few sync-wait slots
    (DMA_DIRECT2D effectively 1, engine ops ~2); Tile can emit more. Move the
    excess onto an EventSemaphore wait on the issuing engine right before the
    instruction (engine streams are in-order, so this is a conservative,
    correct strengthening)."""
    import bass_rust
    fn = nc.m.functions[0]
    for blk in fn.blocks:
        il = list(blk.instructions)
        out = []
        changed = False
        for inst in il:
            tn = type(inst).__name__
            si = inst.sync_info
            if si is not None and tn != "InstEventSemaphore":
                limit = _DMA_WAIT_LIMIT if ("DMA" in tn or "Dma" in tn) \
                    else _ENGINE_WAIT_LIMIT
                w = list(si.on_wait)
                if len(w) > limit:
                    excess = w[:-limit]
                    # EventSemaphore carries <=2 waits and <=1 update; chain
                    # as many as needed, each ticking the dummy wsplit sem.
                    for gi in range(0, len(excess), 2):
                        nop = mybir.InstEventSemaphore(
                            name=f"wsplit{gi}_{inst.name}", ins=[], outs=[])
                        nop.engine = inst.engine
                        nop.sync_info = bass_rust.SyncInfo(
                            on_wait=excess[gi:gi + 2],
                            on_update=[bass_rust.SyncUpdate(
                                sync_type="semaphore", id=wsem.num,
                                ant_name=wsem.name, update_mode="sem-add-imm",
                                update_value=1)])
                        out.append(nop)
                    si.on_wait = w[-limit:]
                    changed = True
            out.append(inst)
        if changed:
            blk.instructions = out


_NC_CACHE = {}


def build_nc(reps=1):
    global _NC_CACHE
    if reps in _NC_CACHE:
        return _NC_CACHE[reps]
    nc = bass.Bass("TRN2", target_bir_lowering=False, debug=False,
                   num_devices=NCORES)
    io = {}
    io["xq"] = nc.dram_tensor("xq", [NQ, C], FP, kind="ExternalInput").ap()
    io["xk"] = nc.dram_tensor("xk", [N, C], FP, kind="ExternalInput").ap()
    io["xv"] = nc.dram_tensor("xv", [N, C], FP, kind="ExternalInput").ap()
    io["M"] = nc.dram_tensor("M", [C, C], BF, kind="ExternalInput").ap()
    io["U"] = nc.dram_tensor("U", [C, C], BF, kind="ExternalInput").ap()
    io["v0"] = nc.dram_tensor("v0", [C], BF, kind="ExternalInput").ap()
    io["bo"] = nc.dram_tensor("bo", [C], FP, kind="ExternalInput").ap()
    io["rscr"] = nc.dram_tensor("rscr", [NQ], FP, kind="Internal").ap()
    io["out"] = nc.dram_tensor("out", [C, NQ], FP, kind="ExternalOutput").ap()

    wsem = nc.alloc_semaphore("wsplit")
    from contextlib import ExitStack
    with tile.TileContext(nc) as tc, ExitStack() as cctx:
        cst = _emit_consts(nc, tc, cctx, io)
        for _ in range(reps):
            _emit(nc, tc, io, cst)
    _split_dma_waits(nc, wsem)
    _NC_CACHE[reps] = nc
    return nc


def make_in_maps(q, k, v, ln_g, ln_b, Wq, bq, Wk, bk, Wv, bv, Wo, bo):
    bf = ml_dtypes.bfloat16
    f8 = np.float64
    g = np.asarray(ln_g, f8)
    be = np.asarray(ln_b, f8)
    Wq_, Wk_, Wv_, Wo_ = (np.asarray(W, f8) for W in (Wq, Wk, Wv, Wo))
    bq_, bv_, bo_ = (np.asarray(x, f8) for x in (bq, bv, bo))
    Wqp = g[:, None] * Wq_
    Wkp = g[:, None] * Wk_
    Wvp = g[:, None] * Wv_
    bqp = be @ Wq_ + bq_
    shared = {
        "M": (Wqp @ Wkp.T).astype(np.float32).astype(bf),
        "U": (Wvp @ Wo_).astype(np.float32).astype(bf),
        "v0": (SCALE * (Wkp @ bqp)).astype(np.float32).astype(bf),
        "bo": (bo_ + (be @ Wv_ + bv_) @ Wo_).astype(np.float32),
    }
    in_maps = []
    for core in range(NCORES):
        b, h = core // 2, core % 2
        m = dict(shared)
        m["xq"] = np.ascontiguousarray(q[b, h * NQ:(h + 1) * NQ, :], np.float32)
        m["xk"] = np.ascontiguousarray(k[b], np.float32)
        m["xv"] = np.ascontiguousarray(v[b], np.float32)
        in_maps.append(m)
    return in_maps


def kernel(q, k, v, ln_g, ln_b, Wq, bq, Wk, bk, Wv, bv, Wo, bo, **run_kwargs):
    nc = build_nc()
    in_maps = make_in_maps(q, k, v, ln_g, ln_b, Wq, bq, Wk, bk, Wv, bv, Wo, bo)
    try:
        res = run_bass_kernel_spmd(nc, in_maps, core_ids=list(range(NCORES)),
                                   **run_kwargs)
    except Exception:
        # transient axon-tunnel failures happen; one retry
        res = run_bass_kernel_spmd(nc, in_maps, core_ids=list(range(NCORES)),
                                   **run_kwargs)
    out = np.empty((B, N, C), np.float32)
    for core in range(NCORES):
        b, h = core // 2, core % 2
        out[b, h * NQ:(h + 1) * NQ, :] = res.results[core]["out"].T
    if run_kwargs:
        kernel.last_results = res
    return out


# revision 30
# speedup vs baseline: 1.6005x; 1.6005x over previous
"""Trainium2 Bass kernel for nn_Attention (dense transformer block, full-dim attention).

Reference computation (per batch b):
    qn/kn/vn = LayerNorm(q/k/v[b])           # over C=256
    qp = qn @ Wq + bq; kp = kn @ Wk + bk; vp = vn @ Wv + bv   # [N, 1024]
    S  = qp @ kp.T * 64^-0.5; P = softmax(S); out = (P @ vp) @ Wo + bo

Key observation: the INNER=1024 dim only appears inside two weight-weight
products, so the whole block is rank-256 through the attention:
    S   = x^q (Wq' Wk'^T) x^k.T + [q-only] + w_k + [const]
    out = P x^v (Wv' Wo) / rowsum + bo''
where x^ = (x-mu)*rstd (pure LN), Wq' = diag(g)Wq etc. Host precomputes
    M = Wq' Wk'^T  [256,256]     U = Wv' Wo  [256,256]
    v0 = SCALE * Wk' @ (beta Wq + bq)        (the k-dependent bias row)
    bo'' = bo + (beta Wv + bv) @ Wo          (exact: softmax rows sum to 1)
The q-only and constant S terms cancel in softmax and are dropped; w_k
rides the ACT exp eviction as a per-partition bias. The 1024-dim
projections, Wq/Wk/Wv/Wo streaming, and their SBUF residency all vanish:
per-core matmul work drops from ~944 to ~256 instructions.

Sharding: 8 cores = 4 batches x 2 query-row halves; k/v LN is duplicated
within the pair (no projections left to dedup -- not worth a collective).

On-chip dataflow (per core), contraction on partitions, operands bf16:
    x^qT, x^kT : LN stats natural (batched bn_stats), PE-transpose
    x^v        : LN apply straight to natural bf16 (no transpose)
    AT  [c',q] : M chunks (lhsT) x x^qT          (8 MMs)
    w   [1,N]  : v0 1-col chunks (lhsT) x x^kT   (8 MMs), DRAM-bounce
                 scatter to [128, MT] per-partition layout
    S^T        : x^kT tiles (lhsT) x AT          (64 MMs, FD=512)
    expS^T     : ACT exp(0.125*S^T + w_m) from PSUM, bf16
    rowsum     : ones [128,128] (lhsT) x expS -> every partition holds the
                 k-sum; reciprocal in free-dim layout      (32 MMs)
    Y^T [c,q]  : x^v tiles (lhsT) x expS^T       (64 MMs)
    out^T      : U chunks (lhsT) x Y^T           (8 MMs), eviction
                 multiplies by recipF (per-free) and adds bo'' (per-part);
                 out stored transposed [C, NQ], host transposes back.

Phase order keeps all ACT Sqrt ops before all Exp ops (activation table
switches cost ~2.7us). A short identity-transpose burst at kernel start
bridges the LN startup bubble and keeps the PE HAM clock gate released.
A post-scheduling pass splits multi-wait instructions (this walrus's
instruction structs carry at most 1-2 sync waits)."""

import numpy as np
import ml_dtypes

import concourse.bass as bass
import concourse.tile as tile
from concourse import mybir
from concourse.bass_utils import run_bass_kernel_spmd

# Problem shapes (hardcoded per contract)
B = 4
N = 2048          # sequence length (k/v tokens per core)
C = 256           # channels
NQ = 1024         # query rows per core (N/2)
EPS = 1e-5
SCALE = 0.125     # 64 ** -0.5
P = 128

FP = mybir.dt.float32
BF = mybir.dt.bfloat16

NCORES = 8
CCH = C // P          # 2 chunks of the channel dim
MT = N // P           # 16 k-token tiles
QT = NQ // P          # 8 q-token tiles
QCH = NQ // 512       # 2 q-token free chunks
KCH = N // 512        # 4 k-token free chunks

_sub = mybir.AluOpType.subtract
_mult = mybir.AluOpType.mult

WARM = 8   # PE warm-up transposes bridging the LN startup bubble


def _bcast(ap, parts=P):
    # prepend a stride-0 partition dim: [n] -> [parts, n]
    return bass.AP(tensor=ap.tensor, offset=ap.offset,
                   ap=[[0, parts]] + [list(d) for d in ap.ap])


def _emit_consts(nc, tc, ctx, io):
    consts = ctx.enter_context(tc.tile_pool(name="consts", bufs=1))
    pools = dict(
        big=ctx.enter_context(tc.tile_pool(name="big", bufs=1)),
        ln_pool=ctx.enter_context(tc.tile_pool(name="ln", bufs=4)),
        stat=ctx.enter_context(tc.tile_pool(name="stat", bufs=4)),
        temps=ctx.enter_context(tc.tile_pool(name="temps", bufs=3)),
        psum=ctx.enter_context(tc.tile_pool(name="psum", bufs=4, space="PSUM")),
        psum_rs=ctx.enter_context(tc.tile_pool(name="psum_rs", bufs=1, space="PSUM")),
        psum_t=ctx.enter_context(tc.tile_pool(name="psum_t", bufs=2, space="PSUM")),
    )
    # ---- constants (scalar/gpsimd DMA queues: off the load path) --
    # M/U as stationary chunks: [128 (contraction part), chunk, out-cols]
    M_sb = consts.tile([P, CCH, C], BF)
    nc.scalar.dma_start(M_sb, io["M"].rearrange("(c p) n -> p c n", p=P))
    U_sb = consts.tile([P, CCH, C], BF)
    nc.scalar.dma_start(U_sb, io["U"].rearrange("(c p) n -> p c n", p=P))
    v0_sb = consts.tile([P, CCH], BF)
    nc.scalar.dma_start(v0_sb, io["v0"].rearrange("(c p) -> p c", p=P))
    ones1 = consts.tile([P, 1], BF)
    nc.vector.memset(ones1, 1.0)
    eps_sb = consts.tile([P, 1], FP)
    nc.vector.memset(eps_sb, EPS)
    ident = consts.tile([P, P], BF)
    from concourse.masks import make_identity
    make_identity(nc, ident)
    # bo as a 1-partition row (bf16): rank-1 matmul folds the bias into
    # the out^T accumulation pre-normalization (bo * rowsum)
    b_row = consts.tile([1, C], BF)
    nc.gpsimd.dma_start(b_row, io["bo"])
    return dict(M_sb=M_sb, U_sb=U_sb, v0_sb=v0_sb, ones1=ones1,
                eps_sb=eps_sb, ident=ident, b_row=b_row, pools=pools)


def _emit(nc, tc, io, cst):
    M_sb = cst["M_sb"]; U_sb = cst["U_sb"]; v0_sb = cst["v0_sb"]
    ones1 = cst["ones1"]; eps_sb = cst["eps_sb"]; ident = cst["ident"]
    b_row = cst["b_row"]
    pl = cst["pools"]
    big = pl["big"]; ln_pool = pl["ln_pool"]; stat = pl["stat"]
    temps = pl["temps"]; psum = pl["psum"]; psum_rs = pl["psum_rs"]
    psum_t = pl["psum_t"]
    if True:
        # ---- persistent activations ----------------------------------
        xqT = big.tile([P, CCH, NQ], BF)
        xkT = big.tile([P, CCH, N], BF)
        xv = big.tile([P, MT, C], BF)
        AT = big.tile([P, CCH, NQ], BF)
        YT = big.tile([P, CCH, NQ], BF)
        expS = big.tile([P, MT, NQ], BF)
        recipF = big.tile([P, NQ], FP)
        recip1 = big.tile([1, NQ], FP)
        rs_sb = big.tile([1, QCH, 512], BF)
        wm_sb = big.tile([P, MT], FP)
        # batched input staging: 4 token-tiles per DMA (amortizes the
        # ~0.65us per-DMA issue cost that dominated the LN front latency)
        xq_l = big.tile([P, QT, C], FP)
        xk_l = big.tile([P, MT, C], FP)
        xv_l = big.tile([P, MT, C], FP)
        for src, dst, nt in ((io["xq"], xq_l, QT), (io["xk"], xk_l, MT),
                             (io["xv"], xv_l, MT)):
            r4 = src.rearrange("(t p) c -> p t c", p=P)
            for t0 in range(0, nt, 4):
                nc.sync.dma_start(dst[:, t0:t0 + 4, :], r4[:, t0:t0 + 4, :])

        # PE warm-up during the LN-chain startup bubble: sustained activity
        # releases the HAM clock gate (1.2 -> 2.4 GHz) before real matmuls
        warm = psum_t.tile([P, P], BF, tag="pst", name="warm")
        for w in range(WARM):
            nc.tensor.transpose(warm, ident, ident)

        # ---- layernorm: stats in natural layout, batched -------------
        def layernorm(x_l, ntiles, dstT=None, dst_nat=None):
            # groups of 8 tiles: batched stats -> one sqrt/recip per group.
            # dstT: apply + PE-transpose (plain eviction, gamma/beta live in
            # the host-folded weights). dst_nat: apply straight to bf16.
            for g0 in range(0, ntiles, 8):
                gn = min(8, ntiles - g0)
                mv_g = stat.tile([P, 8, 2], FP, tag="mv_g")
                xts = []
                for ii in range(gn):
                    i = g0 + ii
                    xt = x_l[:, i, :]
                    st = stat.tile([P, 6], FP, tag="st")
                    nc.vector.bn_stats(st, xt)
                    nc.vector.bn_aggr(mv_g[:, ii, :], st)
                    xts.append(xt)
                rstd_g = stat.tile([P, 8], FP, tag="rstd_g")
                nc.scalar.activation(rstd_g[:, :gn], mv_g[:, :gn, 1],
                                     mybir.ActivationFunctionType.Sqrt,
                                     bias=eps_sb, scale=1.0)
                nc.vector.reciprocal(rstd_g[:, :gn], rstd_g[:, :gn])
                # negated mu*rstd so half the applies can ride ACT's
                # scale/bias path: x*rstd + (-mu*rstd)
                nmr_g = stat.tile([P, 8], FP, tag="nmr_g")
                nc.vector.scalar_tensor_tensor(nmr_g[:, :gn], mv_g[:, :gn, 0],
                                               -1.0, rstd_g[:, :gn],
                                               op0=_mult, op1=_mult)
                for ii in range(gn):
                    i = g0 + ii
                    if dst_nat is not None:
                        dst = dst_nat[:, i, :]
                        if ii % 2 == 0:
                            nc.vector.tensor_scalar(dst, xts[ii],
                                                    mv_g[:, ii, 0:1],
                                                    rstd_g[:, ii:ii + 1],
                                                    op0=_sub, op1=_mult)
                        else:
                            nc.scalar.activation(
                                dst, xts[ii],
                                mybir.ActivationFunctionType.Identity,
                                bias=nmr_g[:, ii:ii + 1],
                                scale=rstd_g[:, ii:ii + 1])
                        continue
                    xn = ln_pool.tile([P, C], BF, tag="xn")
                    if ii % 2 == 0:
                        nc.vector.tensor_scalar(xn, xts[ii], mv_g[:, ii, 0:1],
                                                rstd_g[:, ii:ii + 1],
                                                op0=_sub, op1=_mult)
                    else:
                        nc.scalar.activation(
                            xn, xts[ii],
                            mybir.ActivationFunctionType.Identity,
                            bias=nmr_g[:, ii:ii + 1],
                            scale=rstd_g[:, ii:ii + 1])
                    for c in range(CCH):
                        pst = psum_t.tile([P, P], BF, tag="pst")
                        nc.tensor.transpose(pst, xn[:, c * P:(c + 1) * P],
                                            ident)
                        dst = dstT[:, c, i * P:(i + 1) * P]
                        if (i + c) % 2 == 0:
                            nc.vector.tensor_copy(dst, pst)
                        else:
                            nc.scalar.copy(dst, pst)

        # ---- phase 1: LN(q), A^T = M-chunks x x^qT -------------------
        layernorm(xq_l, QT, dstT=xqT)
        for jp in range(CCH):
            for n in range(QCH):
                ps = psum.tile([P, 512], FP, tag="ps")
                for cc in range(CCH):
                    nc.tensor.matmul(ps,
                                     lhsT=M_sb[:, cc, jp * P:(jp + 1) * P],
                                     rhs=xqT[:, cc, n * 512:(n + 1) * 512],
                                     start=(cc == 0), stop=(cc == CCH - 1))
                d = AT[:, jp, n * 512:(n + 1) * 512]
                if (jp + n) % 2 == 0:
                    nc.vector.tensor_copy(d, ps)
                else:
                    nc.scalar.copy(d, ps)

        # ---- phase 2: LN(k), LN(v) -----------------------------------
        layernorm(xk_l, MT, dstT=xkT)
        layernorm(xv_l, MT, dst_nat=xv)
        # all ACT Sqrt ops are done; prewarm the Exp activation table now so
        # the ~2.7us table load overlaps the k-transpose/wm matmuls instead
        # of stalling the first S eviction
        dummy = stat.tile([P, 1], FP, tag="dummy")
        nc.scalar.activation(dummy, eps_sb,
                             mybir.ActivationFunctionType.Exp, scale=1.0)

        # ---- phase 3: w column, S^T, exp -----------------------------
        # w = x^k @ v0 (k-dependent exp bias) computed directly in the
        # per-partition layout the exp bias needs: x^kT tiles as stationary,
        # v0 chunk as a 1-wide moving operand -> wm[128 k, m] in one bank
        wm_ps = psum_rs.tile([P, MT], FP, tag="wmps")
        for m in range(MT):
            for cc in range(CCH):
                nc.tensor.matmul(wm_ps[:, m:m + 1],
                                 lhsT=xkT[:, cc, m * P:(m + 1) * P],
                                 rhs=v0_sb[:, cc:cc + 1],
                                 start=(cc == 0), stop=(cc == CCH - 1))
        nc.vector.tensor_copy(wm_sb, wm_ps)

        for m in range(MT):
            for n in range(QCH):
                ps = psum.tile([P, 512], FP, tag="ps")
                for cc in range(CCH):
                    nc.tensor.matmul(ps,
                                     lhsT=xkT[:, cc, m * P:(m + 1) * P],
                                     rhs=AT[:, cc, n * 512:(n + 1) * 512],
                                     start=(cc == 0), stop=(cc == CCH - 1))
                nc.scalar.activation(expS[:, m, n * 512:(n + 1) * 512], ps,
                                     mybir.ActivationFunctionType.Exp,
                                     bias=wm_sb[:, m:m + 1], scale=SCALE)

        # ---- phase 4: rowsums + Y^T = x^v-tiles x expS^T -------------
        # rowsums: 1-col ones stationary (LDW ~free), result on partition 0;
        # the two q-chunks share one PSUM bank, with Y matmul groups
        # interleaved so the bank's WAR wait (reciprocal eviction) is hidden
        def rowsum(n):
            rsb = psum_rs.tile([1, 512], FP, tag="rs", name="rsb")
            for m in range(MT):
                nc.tensor.matmul(rsb,
                                 lhsT=ones1,
                                 rhs=expS[:, m, n * 512:(n + 1) * 512],
                                 start=(m == 0), stop=(m == MT - 1))
            nc.vector.reciprocal(recip1[:, n * 512:(n + 1) * 512], rsb)
            nc.scalar.copy(rs_sb[:, n, :], rsb)
            # broadcast the reciprocal row to all partitions (DRAM bounce);
            # runs during the Y matmuls, ready before the out^T evictions
            nc.sync.dma_start(io["rscr"][n * 512:(n + 1) * 512],
                              recip1[0:1, n * 512:(n + 1) * 512])
            nc.gpsimd.dma_start(
                recipF[:, n * 512:(n + 1) * 512],
                _bcast(io["rscr"][n * 512:(n + 1) * 512]))

        def ygroup(j, n):
            ps = psum.tile([P, 512], FP, tag="ps")
            for m in range(MT):
                nc.tensor.matmul(ps,
                                 lhsT=xv[:, m, j * P:(j + 1) * P],
                                 rhs=expS[:, m, n * 512:(n + 1) * 512],
                                 start=(m == 0), stop=(m == MT - 1))
            d = YT[:, j, n * 512:(n + 1) * 512]
            if (j + n) % 2 == 0:
                nc.vector.tensor_copy(d, ps)
            else:
                nc.scalar.copy(d, ps)

        # out^T = U-chunks x Y^T; a rank-1 matmul adds bo * rowsum inside
        # the accumulation (division by rowsum then yields +bo exactly), so
        # the eviction is a single per-free multiply by recipF
        def outgroup(ci, n):
            ps = psum.tile([P, 512], FP, tag="ps")
            for cc in range(CCH):
                nc.tensor.matmul(ps,
                                 lhsT=U_sb[:, cc, ci * P:(ci + 1) * P],
                                 rhs=YT[:, cc, n * 512:(n + 1) * 512],
                                 start=(cc == 0), stop=False)
            nc.tensor.matmul(ps,
                             lhsT=b_row[:, ci * P:(ci + 1) * P],
                             rhs=rs_sb[:, n, :],
                             start=False, stop=True)
            o1 = temps.tile([P, 512], FP, tag="o1")
            nc.vector.tensor_tensor(o1, ps,
                                    recipF[:, n * 512:(n + 1) * 512],
                                    _mult)
            dma = nc.sync if (ci + n) % 2 == 0 else nc.scalar
            dma.dma_start(
                io["out"][ci * P:(ci + 1) * P, n * 512:(n + 1) * 512], o1)

        # interleave: out(*, n=0) runs while Y(*, n=1) streams on the PE
        rowsum(0)
        ygroup(0, 0)
        rowsum(1)
        ygroup(1, 0)
        outgroup(0, 0)
        outgroup(1, 0)
        ygroup(0, 1)
        ygroup(1, 1)
        outgroup(0, 1)
        outgroup(1, 1)


_DMA_WAIT_LIMIT = 1
_ENGINE_WAIT_LIMIT = 1


def _split_dma_waits(nc, wsem):
    """This walrus's instruction structs carry very few sync-wait slots
    (DMA_DIRECT2D effectively 1, engine ops ~2); Tile can emit more. Move the
    excess onto an EventSemaphore wait on the issuing engine right before the
    instruction (engine streams are in-order, so this is a conservative,
    correct strengthening)."""
    import bass_rust
    fn = nc.m.functions[0]
    for blk in fn.blocks:
        il = list(blk.instructions)
        out = []
        changed = False
        for inst in il:
            tn = type(inst).__name__
            si = inst.sync_info
            if si is not None and tn != "InstEventSemaphore":
                limit = _DMA_WAIT_LIMIT if ("DMA" in tn or "Dma" in tn) \
                    else _ENGINE_WAIT_LIMIT
                w = list(si.on_wait)
                if len(w) > limit:
                    excess = w[:-limit]
                    # EventSemaphore carries <=2 waits and <=1 update; chain
                    # as many as needed, each ticking the dummy wsplit sem.
                    for gi in range(0, len(excess), 2):
                        nop = mybir.InstEventSemaphore(
                            name=f"wsplit{gi}_{inst.name}", ins=[], outs=[])
                        nop.engine = inst.engine
                        nop.sync_info = bass_rust.SyncInfo(
                            on_wait=excess[gi:gi + 2],
                            on_update=[bass_rust.SyncUpdate(
                                sync_type="semaphore", id=wsem.num,
                                ant_name=wsem.name, update_mode="sem-add-imm",
                                update_value=1)])
                        out.append(nop)
                    si.on_wait = w[-limit:]
                    changed = True
            out.append(inst)
        if changed:
            blk.instructions = out


_NC_CACHE = {}


def build_nc(reps=1):
    global _NC_CACHE
    if reps in _NC_CACHE:
        return _NC_CACHE[reps]
    nc = bass.Bass("TRN2", target_bir_lowering=False, debug=False,
                   num_devices=NCORES)
    io = {}
    io["xq"] = nc.dram_tensor("xq", [NQ, C], FP, kind="ExternalInput").ap()
    io["xk"] = nc.dram_tensor("xk", [N, C], FP, kind="ExternalInput").ap()
    io["xv"] = nc.dram_tensor("xv", [N, C], FP, kind="ExternalInput").ap()
    io["M"] = nc.dram_tensor("M", [C, C], BF, kind="ExternalInput").ap()
    io["U"] = nc.dram_tensor("U", [C, C], BF, kind="ExternalInput").ap()
    io["v0"] = nc.dram_tensor("v0", [C], BF, kind="ExternalInput").ap()
    io["bo"] = nc.dram_tensor("bo", [C], FP, kind="ExternalInput").ap()
    io["rscr"] = nc.dram_tensor("rscr", [NQ], FP, kind="Internal").ap()
    io["out"] = nc.dram_tensor("out", [C, NQ], FP, kind="ExternalOutput").ap()

    wsem = nc.alloc_semaphore("wsplit")
    from contextlib import ExitStack
    with tile.TileContext(nc) as tc, ExitStack() as cctx:
        cst = _emit_consts(nc, tc, cctx, io)
        for _ in range(reps):
            _emit(nc, tc, io, cst)
    _split_dma_waits(nc, wsem)
    _NC_CACHE[reps] = nc
    return nc


def make_in_maps(q, k, v, ln_g, ln_b, Wq, bq, Wk, bk, Wv, bv, Wo, bo):
    bf = ml_dtypes.bfloat16
    f8 = np.float64
    g = np.asarray(ln_g, f8)
    be = np.asarray(ln_b, f8)
    Wq_, Wk_, Wv_, Wo_ = (np.asarray(W, f8) for W in (Wq, Wk, Wv, Wo))
    bq_, bv_, bo_ = (np.asarray(x, f8) for x in (bq, bv, bo))
    Wqp = g[:, None] * Wq_
    Wkp = g[:, None] * Wk_
    Wvp = g[:, None] * Wv_
    bqp = be @ Wq_ + bq_
    shared = {
        "M": (Wqp @ Wkp.T).astype(np.float32).astype(bf),
        "U": (Wvp @ Wo_).astype(np.float32).astype(bf),
        "v0": (SCALE * (Wkp @ bqp)).astype(np.float32).astype(bf),
        "bo": (bo_ + (be @ Wv_ + bv_) @ Wo_).astype(np.float32),
    }
    in_maps = []
    for core in range(NCORES):
        b, h = core // 2, core % 2
        m = dict(shared)
        m["xq"] = np.ascontiguousarray(q[b, h * NQ:(h + 1) * NQ, :], np.float32)
        m["xk"] = np.ascontiguousarray(k[b], np.float32)
        m["xv"] = np.ascontiguousarray(v[b], np.float32)
        in_maps.append(m)
    return in_maps


def kernel(q, k, v, ln_g, ln_b, Wq, bq, Wk, bk, Wv, bv, Wo, bo, **run_kwargs):
    nc = build_nc()
    in_maps = make_in_maps(q, k, v, ln_g, ln_b, Wq, bq, Wk, bk, Wv, bv, Wo, bo)
    try:
        res = run_bass_kernel_spmd(nc, in_maps, core_ids=list(range(NCORES)),
                                   **run_kwargs)
    except Exception:
        # transient axon-tunnel failures happen; one retry
        res = run_bass_kernel_spmd(nc, in_maps, core_ids=list(range(NCORES)),
                                   **run_kwargs)
    out = np.empty((B, N, C), np.float32)
    for core in range(NCORES):
        b, h = core // 2, core % 2
        out[b, h * NQ:(h + 1) * NQ, :] = res.results[core]["out"].T
    if run_kwargs:
        kernel.last_results = res
    return out
